# revision 31
# baseline (speedup 1.0000x reference)
"""Trainium2 Bass kernel for a 2-layer GCN (DGL GraphConv, norm='both').

Reference computation (per layer):
    h = relu( deg_in^-0.5 * segment_sum( ((x * deg_out^-0.5) @ W)[src], dst ) + b )
then logits = h2 @ Wc + bc.

Distribution: nodes are relabeled into 128-wide blocks, blocks are
load-balanced across the 8 NeuronCores (snake assignment by edge count).
Messages are single bf16 (256B rows); the f32 segment-sum runs in PSUM.

Layer dataflow per core:
  stage A: g = (x @ W) * s_out for the core's shard, written bf16 to the
    g table. Table rows are quarter-major: each core's shard is split in
    4 quarters of 25 blocks, and the global table groups quarter q of
    every core together, so the AllGather is 4 independent per-quarter
    collectives that fire as soon as that quarter's blocks are written
    (overlapping the collectives with compute instead of one barrier).
    Within a quarter, block pairs are row-interleaved (row = 256*(b//2)
    + 2*pos + b%2) so one table-write DMA covers two blocks with
    512B-per-partition descriptors (sub-512B descriptors pay a 2x DMA
    latency penalty).
  stage B: source-chunk window c = quarter c of the table (25600 rows).
    Supergroups of SG=20 destination blocks; edge slots are laid out
    chunk-major: per (supergroup, chunk) run, ring-sized dma_gather
    calls pull all edge messages (slot-level cross-core caps). Per
    (block, chunk) piece one DVE is_equal (paired-loc layout so every
    operand's last AP dim is packed -> 2x DVE mode; iota offset by block
    parity disambiguates shared boundary subtiles) builds the one-hot,
    then per-subtile matmuls accumulate into the block's 128-column
    slice of a shared PSUM bank (4 blocks per 2KB bank, 5 banks per
    supergroup, alive across 4 chunks).
  epilogue: the matmuls accumulate the TRANSPOSED aggregate directly
    (lhsT = messages, rhs = one-hot), so no PE transpose is needed; a
    per-bank Act relu produces h^T; the s_in scale commutes through the
    (zero-bias) relu and folds into the next stage-A scale, and for the
    classifier the s_in scale and bc bias are applied on the host.
    Layer-2 classifier: logits^T = Wc^T h2^T, wide matmuls per PSUM
    bank, written from PSUM.

All index preprocessing (degrees, slot layout, int16 gather indices) is
host-side numpy on integer graph structure; float math is on device.
"""
from dataclasses import dataclass

import numpy as np

import concourse.bacc as bacc
import concourse.mybir as mybir
import concourse.tile as tile
from concourse.bass_utils import run_bass_kernel_spmd

f32 = mybir.dt.float32
bf16 = mybir.dt.bfloat16
i16 = mybir.dt.int16

P = 128  # partitions / node block size

import ml_dtypes  # noqa: E402  (ships with jax)

np_bf16 = ml_dtypes.bfloat16


@dataclass
class Cfg:
    n_nodes: int = 100000
    in_feats: int = 128
    num_classes: int = 4
    n_cores: int = 8
    nb: int = 100         # node blocks per core
    qb: int = 25          # blocks per quarter (collective granularity)
    sg: int = 20          # blocks per supergroup (5 PSUM banks x 4 cols)

    @property
    def npc(self):        # nodes per core
        return self.nb * P

    @property
    def npad(self):       # padded node count
        return self.n_cores * self.npc

    @property
    def nq(self):         # quarters
        return self.nb // self.qb

    @property
    def chunk(self):      # src window rows = one quarter of all cores
        return self.n_cores * self.qb * P

    @property
    def qloc(self):       # per-core rows in one quarter
        return self.qb * P

    @property
    def n_chunks(self):
        return self.nq

    @property
    def n_sg(self):
        return self.nb // self.sg


CFG = Cfg()
assert CFG.n_sg * CFG.sg == CFG.nb
assert CFG.nq * CFG.qb == CFG.nb
assert CFG.chunk <= 2 ** 15  # int16 gather indices
MAXSUB = 16  # subtiles per dma_gather call (= SWDGE ring capacity)


def rows_of_blockmajor(cfg: Cfg, bm: np.ndarray) -> np.ndarray:
    """Map block-major node ids (new_blk*128 + pos) to g-table rows.

    Table rows are quarter-major across cores; within a (core, quarter)
    block pairs are row-interleaved, except the quarter's odd last block
    which stays plain.
    """
    blk = bm >> 7
    p = bm & 127
    core = blk // cfg.nb
    b = blk % cfg.nb
    q = b // cfg.qb
    bq = b % cfg.qb
    rlq = np.where(bq < cfg.qb - 1,
                   (bq >> 1) * (2 * P) + 2 * p + (bq & 1),
                   (cfg.qb - 1) * P + p)
    return q * cfg.chunk + core * cfg.qloc + rlq


class Geometry:
    """Static slot layout. cap[b, c] = cross-core max edge count of the
    (block b, chunk c) cell, >= P so no subtile spans 3 pieces (parity
    one-hot encoding stays unambiguous). Slots of run (sg, c) are the
    concatenation of the supergroup's pieces, tightly packed, rounded up
    to a whole subtile only at the run end."""

    def __init__(self, cfg: Cfg, cap: np.ndarray):
        self.cfg = cfg
        self.cap = cap  # [nb, n_chunks]
        NSG, NCH, SG = cfg.n_sg, cfg.n_chunks, cfg.sg
        self.piece_off = np.zeros((cfg.nb, NCH), np.int64)
        self.Rrun = np.zeros((NSG, NCH), np.int64)
        self.run_slots = np.zeros((NSG, NCH), np.int64)
        # pieces[g][c] = list of (b, i, off, cnt, s0, span)
        self.pieces = [[[] for _ in range(NCH)] for _ in range(NSG)]
        for g in range(NSG):
            for c in range(NCH):
                off = 0
                for i in range(SG):
                    b = g * SG + i
                    n = int(cap[b, c])
                    self.piece_off[b, c] = off
                    s0 = off // P
                    span = -(-(off % P + n) // P)
                    self.pieces[g][c].append((b, i, off, n, s0, span))
                    off += n
                self.Rrun[g, c] = -(-off // P)
                self.run_slots[g, c] = off
        self.Rmax = int(self.Rrun.max())
        self.maxspan = max(pc[5] for g in range(NSG) for c in range(NCH)
                           for pc in self.pieces[g][c])
        self.n_runs = NSG * NCH


def preprocess(cfg: Cfg, src: np.ndarray, dst: np.ndarray):
    """Relabel nodes, lay edges into chunk-major tightly packed slots.

    Returns (geom, node_new, idx16, loc2):
      idx16: [n_cores, n_runs, P, Rmax*8] int16 (16-wrapped, 8x repl)
      loc2:  [n_cores, n_runs, P, Rmax*2] bf16; each subtile's value
             (dst_local + 128*(i%2), or 1000.0 pad) stored twice so the
             is_equal operands can use a packed-pair last AP dim.
    Run r = sg*n_chunks + chunk.  Slot j -> partition j%128, subtile j//128.
    """
    ncores, nb, nch = cfg.n_cores, cfg.nb, cfg.n_chunks
    CH, QB = cfg.chunk, cfg.qb
    SG, NSG = cfg.sg, cfg.n_sg
    n_blocks = ncores * nb

    # block load balancing: snake-assign blocks by edge count
    blk_tot = np.bincount(dst >> 7, minlength=n_blocks)
    order = np.argsort(-blk_tot, kind="stable")
    rank = np.arange(n_blocks)
    lane = rank % ncores
    rev = (rank // ncores) % 2 == 1
    core_of_rank = np.where(rev, ncores - 1 - lane, lane)
    core_of_old = np.empty(n_blocks, np.int64)
    pos_of_old = np.empty(n_blocks, np.int64)
    core_of_old[order] = core_of_rank
    pos_of_old[order] = rank // ncores
    new_blk_of_old = core_of_old * nb + pos_of_old
    node_ar = np.arange(cfg.npad, dtype=np.int64)
    node_new = new_blk_of_old[node_ar >> 7] * P + (node_ar & 127)

    def cell_counts(node_map):
        src_n = node_map[src]
        dst_n = node_map[dst]
        blk = dst_n >> 7
        m_e = blk // nb
        b_e = blk % nb
        c_e = rows_of_blockmajor(cfg, src_n) // CH
        counts = np.bincount((m_e * nb + b_e) * nch + c_e,
                             minlength=n_blocks * nch).reshape(
            ncores, nb, nch)
        return counts

    counts = cell_counts(node_new)

    # refine within-core block->position matching to shrink the
    # cross-core per-(position, chunk) caps (gather padding); swaps are
    # restricted to the same quarter so source chunks stay fixed
    perm = np.tile(np.arange(nb), (ncores, 1))
    cperm = counts.copy()
    for _ in range(8):
        swaps = 0
        for m in range(ncores):
            others = np.delete(cperm, m, axis=0)
            mo = np.maximum(others.max(axis=0), P).astype(np.int64)
            base = np.maximum(mo, cperm[m]).sum(1)
            for b1 in range(nb):
                v1 = cperm[m, b1]
                new1 = np.maximum(mo[b1][None, :], cperm[m]).sum(1)
                new2 = np.maximum(mo, v1[None, :]).sum(1)
                delta = (new1 + new2) - (base[b1] + base)
                delta[b1] = 0
                # same-quarter swaps only
                qmask = np.arange(nb) // QB != b1 // QB
                delta[qmask] = 0
                b2 = int(np.argmin(delta))
                if delta[b2] < 0:
                    cperm[m, [b1, b2]] = cperm[m, [b2, b1]]
                    perm[m, [b1, b2]] = perm[m, [b2, b1]]
                    base = np.maximum(mo, cperm[m]).sum(1)
                    swaps += 1
        if not swaps:
            break
    # perm[m, p_new] = p_old; apply to the relabeling
    newpos_of_pos = np.empty((ncores, nb), np.int64)
    for m in range(ncores):
        newpos_of_pos[m, perm[m]] = np.arange(nb)
    pos_of_old = newpos_of_pos[core_of_old, pos_of_old]
    new_blk_of_old = core_of_old * nb + pos_of_old
    node_new = new_blk_of_old[node_ar >> 7] * P + (node_ar & 127)
    counts = cell_counts(node_new)
    # cperm is bookkeeping; counts recomputed from the final labeling is
    # the ground truth for the slot geometry either way
    if not (counts == cperm).all():
        pass

    src_n = node_new[src]
    dst_n = node_new[dst]
    src_row = rows_of_blockmajor(cfg, src_n)
    blk = dst_n >> 7
    m_e = blk // nb
    b_e = blk % nb
    c_e = src_row // CH

    cap = np.maximum(counts.max(axis=0), P)   # [nb, nch]
    geom = Geometry(cfg, cap)

    # sort edges by (core, supergroup, chunk, block, src)
    g_e = b_e // SG
    k = ((m_e * NSG + g_e) * nch + c_e) * nb + b_e
    perm_e = np.lexsort((src_row, k))
    row_s = src_row[perm_e]
    m_s = m_e[perm_e]
    b_s = b_e[perm_e]
    c_s = c_e[perm_e]
    g_s = g_e[perm_e]
    i_s = b_s - g_s * SG
    loc_s = (dst_n[perm_e] & 127) + P * (i_s % 2)

    kcum = np.zeros(ncores * NSG * nch * nb + 1, np.int64)
    np.cumsum(np.bincount(k[perm_e], minlength=ncores * NSG * nch * nb),
              out=kcum[1:])
    within = np.arange(len(row_s)) - kcum[k[perm_e]]
    slot = geom.piece_off[b_s, c_s] + within
    run_s = g_s * nch + c_s

    Rmax = geom.Rmax
    loc2 = np.full((ncores, geom.n_runs, P, Rmax, 2), 1000.0, np_bf16)
    lv = loc_s.astype(np_bf16)
    loc2[m_s, run_s, slot % P, slot // P, 0] = lv
    loc2[m_s, run_s, slot % P, slot // P, 1] = lv
    loc2 = loc2.reshape(ncores, geom.n_runs, P, Rmax * 2)

    val = (row_s - c_s * CH).astype(np.int16)
    flat = np.zeros((ncores, geom.n_runs, 16, Rmax * 8), np.int16)
    flat[m_s, run_s, slot % 16, slot // 16] = val
    idx16 = np.tile(flat, (1, 1, 8, 1))
    return geom, node_new, idx16, loc2


def build_program(cfg: Cfg, geom: Geometry, single_core_sim=False):
    F = cfg.in_feats
    NB, NPC, NPAD = cfg.nb, cfg.npc, cfg.npad
    NCH, CH = cfg.n_chunks, cfg.chunk
    QB, QLOC = cfg.qb, cfg.qloc
    NSG, SG = cfg.n_sg, cfg.sg
    NCLS = cfg.num_classes
    Rmax, maxspan = geom.Rmax, geom.maxspan
    NRUN = geom.n_runs
    NBANK = -(-SG // 4)  # PSUM banks per supergroup

    n_dev = 1 if single_core_sim else cfg.n_cores
    # 32KB/partition SWDGE scratch = 2048-descriptor ring so each gather
    # call can cover 16 subtiles (the 994ns fixed Q7 cost per call was
    # ~60% of Pool-engine time at the default 1024-descriptor ring)
    nc = bacc.Bacc("TRN2", target_bir_lowering=False, debug=False,
                   num_devices=n_dev, dynamic_dma_scratch_size=32768)

    xT = nc.declare_dram_parameter("xT", [F, NPC], bf16, isOutput=False)
    W1 = nc.declare_dram_parameter("W1", [F, F], bf16, isOutput=False)
    W2 = nc.declare_dram_parameter("W2", [F, F], bf16, isOutput=False)
    Wc = nc.declare_dram_parameter("Wc", [F, NCLS], bf16, isOutput=False)
    souts1 = nc.declare_dram_parameter("souts1", [P, NB], f32,
                                       isOutput=False)
    souts2 = nc.declare_dram_parameter("souts2", [P, NB], f32,
                                       isOutput=False)
    idx16 = nc.declare_dram_parameter("idx16", [NRUN, P, Rmax * 8], i16,
                                      isOutput=False)
    locm = nc.declare_dram_parameter("locm", [NRUN, P, Rmax * 2], bf16,
                                     isOutput=False)
    iota_w = nc.declare_dram_parameter("iota_w", [2, P, maxspan * P], bf16,
                                       isOutput=False)
    logitsT = nc.declare_dram_parameter("logitsT", [NCLS, NPC], f32,
                                        isOutput=True)

    with tile.TileContext(nc) as tc:
        with (
            tc.tile_pool(name="dram", bufs=1, space="DRAM") as dram,
            tc.tile_pool(name="consts", bufs=1) as consts,
            tc.tile_pool(name="meta", bufs=1) as metap,
            tc.tile_pool(name="hT", bufs=1) as hTp,
            tc.tile_pool(name="lhs", bufs=2) as lhsp,
            tc.tile_pool(name="gsg", bufs=8) as gsgp,
            tc.tile_pool(name="gat", bufs=4) as gatp,
            tc.tile_pool(name="oh", bufs=8) as ohp,
            tc.tile_pool(name="hsl", bufs=3) as hslp,
            tc.tile_pool(name="out", bufs=4) as outp,
            tc.tile_pool(name="psA", bufs=2, space="PSUM") as psA,
            tc.tile_pool(name="psB", bufs=1, space="PSUM") as psB,
        ):
            g_loc = dram.tile([NPC, F], bf16, name="g_loc")
            aspace = "Local" if single_core_sim else "Shared"
            # one Shared tile per quarter window: each is written by
            # exactly one collective (Shared DRAM is single-writer)
            g1_full = [dram.tile([CH, F], bf16, addr_space=aspace,
                                 name=f"g1_full_{q}") for q in range(NCH)]
            g2_full = [dram.tile([CH, F], bf16, addr_space=aspace,
                                 name=f"g2_full_{q}") for q in range(NCH)]

            W1_sb = consts.tile([F, F], bf16, name="W1_sb")
            nc.sync.dma_start(W1_sb[:], W1[:])
            souts1_sb = consts.tile([P, NB], f32, name="souts1_sb")
            nc.sync.dma_start(souts1_sb[:], souts1[:])
            iota_sb = [consts.tile([P, maxspan * P], bf16, name=f"iota{j}")
                       for j in range(2)]
            nc.sync.dma_start(iota_sb[0][:], iota_w[0])
            nc.sync.dma_start(iota_sb[1][:], iota_w[1])
            W2_sb = consts.tile([F, F], bf16, name="W2_sb")
            nc.sync.dma_start(W2_sb[:], W2[:])
            Wc_sb = consts.tile([F, NCLS], bf16, name="Wc_sb")
            nc.sync.dma_start(Wc_sb[:], Wc[:])
            souts2_sb = consts.tile([P, NB], f32, name="souts2_sb")
            nc.sync.dma_start(souts2_sb[:], souts2[:])

            # per-run gather metadata tiles, resident across both layers;
            # loads are emitted after the stage-A1 loop (load_meta) so the
            # SP queue feeds stage A first
            idx_sb = []
            loc_sb = []
            for r in range(NRUN):
                g, c = divmod(r, NCH)
                Rr = int(geom.Rrun[g, c])
                idx_sb.append(metap.tile([P, Rr * 8], i16, name=f"idx_{r}"))
                loc_sb.append(metap.tile([P, Rr * 2], bf16,
                                         name=f"loc_{r}"))

            def load_meta(g):
                for c in range(NCH):
                    r = g * NCH + c
                    Rr = int(geom.Rrun[g, c])
                    nc.sync.dma_start(idx_sb[r][:], idx16[r, :, :Rr * 8])
                    nc.sync.dma_start(loc_sb[r][:], locm[r, :, :Rr * 2])

            h1T = [hTp.tile([F, SG * P], bf16, name=f"h1T_{g}")
                   for g in range(NSG)]

            # stage A writes the table in block pairs: the pair's rows are
            # interleaved so one DMA covers both blocks with 512B
            # descriptors; each quarter's odd last block is written plain.
            # After a quarter's last block, its AllGather fires (the 4
            # per-quarter collectives overlap with compute downstream).
            pair_state = {}

            def table_rows(q, lo, hi, g_full):
                if single_core_sim:
                    return g_full[q][lo:hi, :]
                base = q * QLOC
                return g_loc[base + lo:base + hi, :]

            def stage_a(b, lhs_ap, W_sb, scale_sb, g_full, deng):
                q, bq = divmod(b, QB)
                last = bq == QB - 1
                if not last and bq % 2 == 0:
                    pair_state["pa"] = psA.tile([P, 2 * F], f32, name="pa",
                                                tag="pa")
                    nc.tensor.matmul(pair_state["pa"][:, :F], lhs_ap,
                                     W_sb[:], start=True, stop=False)
                    return
                if not last:
                    pa = pair_state["pa"]
                    nc.tensor.matmul(pa[:, F:], lhs_ap, W_sb[:],
                                     start=False, stop=True)
                    gst = gsgp.tile([P, 2 * F], bf16, name="gst", tag="gst")
                    for j in (0, 1):
                        nc.scalar.activation(
                            out=gst[:, j * F:(j + 1) * F],
                            in_=pa[:, j * F:(j + 1) * F],
                            func=mybir.ActivationFunctionType.Copy,
                            scale=scale_sb[:, b - 1 + j:b + j])
                    k = bq // 2
                    dst = table_rows(q, k * 2 * P, (k + 1) * 2 * P, g_full)
                    deng.dma_start(
                        dst.rearrange("(p two) f -> p (two f)", p=P),
                        gst[:])
                    return
                # quarter's last block: plain write, then the collective
                pa = psA.tile([P, F], f32, name="pa1", tag="pa")
                nc.tensor.matmul(pa[:], lhs_ap, W_sb[:], start=True,
                                 stop=True)
                gst = gsgp.tile([P, F], bf16, name="gst1", tag="gst")
                nc.scalar.activation(
                    out=gst[:], in_=pa[:],
                    func=mybir.ActivationFunctionType.Copy,
                    scale=scale_sb[:, b:b + 1])
                dst = table_rows(q, (QB - 1) * P, QB * P, g_full)
                deng.dma_start(dst, gst[:])
                if not single_core_sim:
                    nc.gpsimd.collective_compute(
                        "AllGather", mybir.AluOpType.bypass,
                        replica_groups=[list(range(cfg.n_cores))],
                        ins=[g_loc[q * QLOC:(q + 1) * QLOC, :]],
                        outs=[g_full[q][:]])

            def one_hot(r, i, s0, span):
                oh = ohp.tile([P, maxspan * P], bf16, name="oh", tag="oh")
                in1 = (loc_sb[r][:, s0 * 2:(s0 + span) * 2]
                       .rearrange("p (s e) -> p s e", e=2)
                       .unsqueeze(2).broadcast_to([P, span, 64, 2]))
                nc.vector.tensor_tensor(
                    out=oh[:, :span * P].rearrange(
                        "p (s j e) -> p s j e", s=span, j=64),
                    in0=iota_sb[i % 2][:, :span * P].rearrange(
                        "p (s j e) -> p s j e", s=span, j=64),
                    in1=in1,
                    op=mybir.AluOpType.is_equal)
                return oh

            # chunk c is gathered from quarter-c of the table, available
            # as soon as collective c lands: process chunks in order
            C_ORDER = list(range(NCH))

            def layer_b(layer, g_full, W_next, g_full_next):
                # the epilogue (and the next layer's stage-A writes) of
                # supergroup g-1 is emitted after supergroup g's first
                # gather: HWDGE DMAs complete in FIFO order per engine, so
                # a gather emitted after those SP writes would stall the
                # Pool queue behind them
                pend = [None]

                def epilogue(layer, g, pb):
                    # epilogue: per-bank relu on the Act engine (the
                    # s_in scale and bc add are linear and applied on the
                    # host); layer-2 classifier is transposed, wide
                    # matmuls per PSUM bank, logits DMA'd from PSUM
                    for j in range(NBANK):
                        w = (min(SG, 4 * j + 4) - 4 * j) * P
                        if layer == 1:
                            nc.scalar.activation(
                                out=h1T[g][:, 4 * j * P:4 * j * P + w],
                                in_=pb[j][:, :w],
                                func=mybir.ActivationFunctionType.Relu)
                        else:
                            hsl = hslp.tile([F, 4 * P], bf16, name="hsl",
                                            tag="hsl")
                            nc.scalar.activation(
                                out=hsl[:, :w], in_=pb[j][:, :w],
                                func=mybir.ActivationFunctionType.Relu)
                            pc = psA.tile([NCLS, 4 * P], f32, name="pc",
                                          tag="pa")
                            nc.tensor.matmul(pc[:, :w], Wc_sb[:],
                                             hsl[:, :w],
                                             start=True, stop=True)
                            ow = outp.tile([NCLS, 4 * P], f32,
                                           name="ow", tag="ow")
                            nc.scalar.activation(
                                out=ow[:, :w], in_=pc[:, :w],
                                func=mybir.ActivationFunctionType.Copy)
                            c0j = (g * SG + 4 * j) * P
                            nc.sync.dma_start(
                                logitsT[:, c0j:c0j + w], ow[:, :w])
                    if layer == 1:
                        for i in range(SG):
                            b = g * SG + i
                            # layer-2 table writes ride the Act HWDGE
                            # FIFO so layer-1 gathers' SP-FIFO watermark
                            # waits don't round up over them
                            stage_a(b, h1T[g][:, i * P:(i + 1) * P],
                                    W_next, souts2_sb, g_full_next,
                                    nc.scalar)

                def flush_pend():
                    if pend[0] is not None:
                        epilogue(*pend[0])
                        pend[0] = None

                for g in range(NSG):
                    gg = (layer - 1) * NSG + g
                    # all 4 chunk gathers up-front, then the previous
                    # supergroup's epilogue, then this one's matmuls: the
                    # gathers must precede the epilogue's DMA writes in
                    # queue order or they stall behind them
                    gats = []
                    for c in C_ORDER:
                        r = g * NCH + c
                        Rr = int(geom.Rrun[g, c])
                        gat = gatp.tile([P, Rmax * F], bf16, name="gat",
                                        tag="gat")
                        gats.append(gat)
                        # ring-sized gather calls; pad slots (zero idx)
                        # are gathered too so every tile region the
                        # matmuls read holds finite data
                        for s0 in range(0, Rr, MAXSUB):
                            s1 = min(s0 + MAXSUB, Rr)
                            n_idx = (s1 - s0) * P
                            out_ap = gat[:, s0 * F:s1 * F].rearrange(
                                "p (s f) -> p s f", s=s1 - s0)
                            nc.gpsimd.dma_gather(
                                out_ap=out_ap,
                                in_ap=g_full[c][:],
                                idxs_ap=idx_sb[r][:, s0 * 8:s1 * 8],
                                num_idxs=n_idx,
                                num_idxs_reg=n_idx,
                                elem_size=F,
                                single_packet=False,
                            )
                    flush_pend()
                    pb = [psB.tile(
                        [P, 4 * P], f32, name=f"pb{g}_{j}",
                        tag=f"pb{(gg * NBANK + j) % (NBANK + 1)}")
                        for j in range(NBANK)]
                    for ci, c in enumerate(C_ORDER):
                        r = g * NCH + c
                        gat = gats[ci]
                        for (b, i, off, cnt, s0, span) in geom.pieces[g][c]:
                            oh = one_hot(r, i, s0, span)
                            pbt, col = pb[i // 4], (i % 4) * P
                            # start/stop zero and close the whole 2KB
                            # bank: flag only once per (bank, sg)
                            first_of_bank = i % 4 == 0
                            last_of_bank = (i % 4 == 3 or i == SG - 1)
                            for t in range(s0, s0 + span):
                                nc.tensor.matmul(
                                    pbt[:, col:col + P],
                                    gat[:, t * F:(t + 1) * F],
                                    oh[:, (t - s0) * P:(t - s0 + 1) * P],
                                    start=(ci == 0 and t == s0
                                           and first_of_bank),
                                    stop=(ci == NCH - 1
                                          and t == s0 + span - 1
                                          and last_of_bank))
                    pend[0] = (layer, g, pb)
                flush_pend()

            xsg = None
            for b in range(NB):
                if b % SG == 0:
                    xsg = lhsp.tile([F, SG * P], bf16, name="xsg",
                                    tag="xsg")
                    nc.sync.dma_start(
                        xsg[:], xT[:, b * P:(b + SG) * P])
                if b % SG == 10:
                    # meta loads sit behind this quarter's table writes in
                    # the SP FIFO; defer them past the write burst so the
                    # per-quarter collectives (and first gathers) fire
                    # sooner
                    load_meta(b // SG)
                i = b % SG
                stage_a(b, xsg[:, i * P:(i + 1) * P], W1_sb, souts1_sb,
                        g1_full, nc.sync)
            layer_b(1, g1_full, W2_sb, g2_full)
            layer_b(2, g2_full, None, None)

    nc.compile()
    return nc


def run(cfg: Cfg, features, src, dst, W1, b1, W2, b2, Wc, bc,
        trace=False, return_results=False):
    F, NPC, NPAD = cfg.in_feats, cfg.npc, cfg.npad
    n = cfg.n_nodes
    src = np.asarray(src).astype(np.int64)
    dst = np.asarray(dst).astype(np.int64)
    features = np.asarray(features, np.float32)
    deg_out = np.bincount(src, minlength=NPAD).astype(np.float32)
    deg_in = np.bincount(dst, minlength=NPAD).astype(np.float32)
    s_out_old = 1.0 / np.sqrt(np.maximum(deg_out, 1.0))
    s_in_old = 1.0 / np.sqrt(np.maximum(deg_in, 1.0))

    assert not np.any(np.asarray(b1)) and not np.any(np.asarray(b2)), \
        "kernel assumes zero hidden biases (relu/s_in commutation)"
    geom, node_new, idx16, loc2 = preprocess(cfg, src, dst)

    x_new = np.zeros((NPAD, F), np.float32)
    x_new[node_new[:n]] = features
    s_out = np.ones(NPAD, np.float32)
    s_out[node_new] = s_out_old
    s_in = np.ones(NPAD, np.float32)
    s_in[node_new] = s_in_old
    xT_full = np.ascontiguousarray(x_new.T)

    iota_np = np.empty((2, P, geom.maxspan * P), np_bf16)
    base = np.tile(np.arange(P, dtype=np.float32), geom.maxspan)
    iota_np[0] = np.tile(base[None, :], (P, 1)).astype(np_bf16)
    iota_np[1] = np.tile((base + P)[None, :], (P, 1)).astype(np_bf16)

    in_maps = []
    for m in range(cfg.n_cores):
        sl = slice(m * NPC, (m + 1) * NPC)
        in_maps.append({
            "xT": np.ascontiguousarray(xT_full[:, sl]).astype(np_bf16),
            "W1": np.asarray(W1, np.float32).astype(np_bf16),
            "W2": np.asarray(W2, np.float32).astype(np_bf16),
            "Wc": np.asarray(Wc, np.float32).astype(np_bf16),
            "souts1": np.ascontiguousarray(
                s_out[sl].reshape(cfg.nb, P).T),
            "souts2": np.ascontiguousarray(
                (s_out * s_in)[sl].reshape(cfg.nb, P).T),
            "idx16": idx16[m],
            "locm": loc2[m],
            "iota_w": iota_np,
        })

    nc = build_program(cfg, geom)
    last_err = None
    for _attempt in range(3):
        try:
            res = run_bass_kernel_spmd(nc, in_maps, list(range(cfg.n_cores)),
                                       trace=trace)
            break
        except Exception as e:  # transient axon worker hiccups
            last_err = e
    else:
        raise last_err
    out_new = np.concatenate([r["logitsT"].T for r in res.results], axis=0)
    out_new = out_new * s_in[:, None] + np.asarray(bc, np.float32)[None, :]
    out = out_new[node_new[:n]].astype(np.float32)
    if return_results:
        return out, res
    return out


def kernel(features, src, dst, W1, b1, W2, b2, Wc, bc):
    return run(CFG, features, src, dst, W1, b1, W2, b2, Wc, bc)
